# revision 25
# baseline (speedup 1.0000x reference)
"""InfiniteHeadAttention Trainium2 kernel (8 NeuronCores).

Reference computation (B=4, T=2048, C=1024, H=16, D=64):
    q,k,v = x@Wq, x@Wk, x@Wv  (per-head split)
    att   = softmax(causal(q k^T / sqrt(D)))
    y     = sum over heads of att@v        # heads SUMMED, not concatenated
    out   = y @ Wp

Sharding: 4-way data-parallel over batch x 2-way over heads.
Core c = 2*b+g handles batch b, heads 8g..8g+7. The per-head y partial sums
are combined with a ReduceScatter over core pairs {2b, 2b+1}; core 2b+g then
applies c_proj to token half g and writes out[b, 1024g:1024(g+1), :].

On-core layout is fully "transposed": projections produce q^T,k^T with the
head dim on partitions, attention computes S^T = K Q^T tiles (128 k-tokens x
512 q-tokens), exp on ScalarE, and P^T V via PSUM accumulation with an extra
ones-column in V producing the softmax denominators for free. Matmuls run in
float32r (1 cycle/row vs 4 for fp32; ~12-13 bit mantissa, plenty for this
problem). Causal masking: lower-triangle k-tiles are skipped outright, the
diagonal tile is masked with a precomputed triangle after exp.
"""

import numpy as np

B, T, C = 4, 2048, 1024
H, D = 16, 64
N_CORES = 8
PAIRS = 4          # head pairs per core (2 heads each)
NCT = C // 128     # c-tiles
NTT = T // 128     # token tiles
NQC = T // 512     # q-chunks
NKT = T // 128     # k-tiles
SCALE = 1.0 / 8.0  # 1/sqrt(D)

_cache = {}


def _build(sim_no_collective=False):
    import concourse.bass as bass
    import concourse.bacc as bacc
    import concourse.tile as tile
    from concourse import mybir
    from concourse.bass import ts, ds
    from concourse.masks import make_identity

    f32, f32r = mybir.dt.float32, mybir.dt.float32r
    Exp = mybir.ActivationFunctionType.Exp

    nc = bacc.Bacc("TRN2", target_bir_lowering=False, debug=False,
                   num_devices=1 if sim_no_collective else N_CORES)

    x_s = nc.dram_tensor("x_s", [T, C], f32r, kind="ExternalInput").ap()
    wq_s = nc.dram_tensor("wq_s", [C, 512], f32r, kind="ExternalInput").ap()
    wk_s = nc.dram_tensor("wk_s", [C, 512], f32r, kind="ExternalInput").ap()
    wv_s = nc.dram_tensor("wv_s", [C, 512], f32r, kind="ExternalInput").ap()
    wp = nc.dram_tensor("wp", [D, C], f32r, kind="ExternalInput").ap()
    out_s = nc.dram_tensor("out_s", [T // 2, C], f32, kind="ExternalOutput").ap()

    with tile.TileContext(nc) as tc:
        with (
            tc.tile_pool(name="const", bufs=1) as const,
            tc.tile_pool(name="xTp", bufs=1) as xTp,
            tc.tile_pool(name="xn", bufs=2) as xn_pool,
            tc.tile_pool(name="wqk", bufs=1) as wqk_pool,
            tc.tile_pool(name="qk", bufs=2) as qk_pool,
            tc.tile_pool(name="pp", bufs=2) as p_pool,
            tc.tile_pool(name="norm", bufs=1) as norm,
            tc.tile_pool(name="co", bufs=2) as co_pool,
            tc.tile_pool(name="dram", bufs=1, space="DRAM") as dram,
            tc.tile_pool(name="ps_s", bufs=1, space="PSUM") as ps_s,
            tc.tile_pool(name="ps_o", bufs=1, space="PSUM") as ps_o,
            tc.tile_pool(name="ps_x", bufs=2, space="PSUM") as ps_x,
        ):
            ident = const.tile([128, 128], f32)
            make_identity(nc, ident)
            identr = const.tile([128, 128], f32r)
            nc.vector.tensor_copy(identr, ident)
            # tri[ik, iq] = 1 if iq >= ik else 0  (valid = k <= q on diagonal tile)
            tri = const.tile([128, 128], f32)
            nc.vector.memset(tri, 1.0)
            nc.gpsimd.affine_select(out=tri, in_=tri,
                                    compare_op=mybir.AluOpType.is_ge,
                                    fill=0.0, base=0, pattern=[[1, 128]],
                                    channel_multiplier=-1)
            ones_c = const.tile([128, 1], f32)
            nc.vector.memset(ones_c, 1.0)

            wp_sb = const.tile([D, C], f32r)
            nc.scalar.dma_start(wp_sb, wp)
            wv_sb = const.tile([128, NCT, 512], f32r)
            nc.scalar.dma_start(wv_sb, wv_s.rearrange("(ct p) d -> p ct d", p=128))

            # v with a ones column appended per head: [tok, kt, head, 65]
            vaug = const.tile([128, NKT, 8, 65], f32r)
            nc.vector.tensor_copy(vaug[:, :, :, 64:65],
                                  ones_c.to_broadcast([128, NKT, 8, 1]))
            yT = const.tile([D, T], f32)

            xT = xTp.tile([128, NCT, T], f32r)

            # ---- Phase 1: transpose x into xT, then v projection per token tile
            for tt in range(NTT):
                xn = xn_pool.tile([128, C], f32r)
                nc.sync.dma_start(xn[:, 0:512], x_s[ts(tt, 128), 0:512])
                nc.scalar.dma_start(xn[:, 512:1024], x_s[ts(tt, 128), 512:1024])
                for half in range(2):
                    tag_ = "s0" if half == 0 else "s1"
                    tp = ps_s.tile([128, 4, 128], f32r, tag=tag_)
                    for ci in range(4):
                        ct = 4 * half + ci
                        nc.tensor.transpose(tp[:, ci, :], xn[:, ts(ct, 128)], identr)
                    if (tt + half) % 2 == 0:
                        nc.scalar.copy(xT[:, ds(4 * half, 4), ts(tt, 128)], tp)
                    else:
                        nc.vector.tensor_copy(xT[:, ds(4 * half, 4), ts(tt, 128)], tp)
                vp = ps_o.tile([128, 512], f32, tag="o0" if tt % 2 == 0 else "o1")
                for ct in range(NCT):
                    nc.tensor.matmul(vp, xT[:, ct, ts(tt, 128)], wv_sb[:, ct, :],
                                     start=(ct == 0), stop=(ct == NCT - 1))
                nc.vector.tensor_copy(vaug[:, tt, :, 0:64],
                                      vp.rearrange("p (h d) -> p h d", h=8))

            # ---- Phase 2: per head-pair projection + attention
            for p in range(PAIRS):
                wq_p = wqk_pool.tile([128, NCT, 128], f32r, tag="wq")
                wk_p = wqk_pool.tile([128, NCT, 128], f32r, tag="wk")
                nc.scalar.dma_start(
                    wq_p, wq_s.rearrange("(ct p) d -> p ct d", p=128)[:, :, ds(128 * p, 128)])
                nc.scalar.dma_start(
                    wk_p, wk_s.rearrange("(ct p) d -> p ct d", p=128)[:, :, ds(128 * p, 128)])
                qT = qk_pool.tile([128, T], f32r, tag="q")
                kT = qk_pool.tile([128, T], f32r, tag="k")
                for tc4 in range(NQC):
                    for w_p, dst in ((wq_p, qT), (wk_p, kT)):
                        pj = ps_x.tile([128, 512], f32, tag="px")
                        for ct in range(NCT):
                            nc.tensor.matmul(pj, w_p[:, ct, :],
                                             xT[:, ct, ts(tc4, 512)],
                                             start=(ct == 0), stop=(ct == NCT - 1))
                        nc.vector.tensor_copy(dst[:, ts(tc4, 512)], pj)

                for qc in range(NQC):
                    o_ps0 = ps_o.tile([65, 512], f32, tag="o0")
                    o_ps1 = ps_o.tile([65, 512], f32, tag="o1")
                    o_ps = [o_ps0, o_ps1]
                    nkt = 4 * qc + 4
                    for g0 in range(0, nkt, 2):
                        m_g0 = g0 - 4 * qc
                        eoff = 256 if m_g0 == 2 else 0
                        s_ps0 = ps_s.tile([128, 2, 512], f32, tag="s0")
                        s_ps1 = ps_s.tile([128, 2, 512], f32, tag="s1")
                        s_ps = [s_ps0, s_ps1]
                        p_sb0 = p_pool.tile([128, 2, 512], f32r, tag="p0")
                        p_sb1 = p_pool.tile([128, 2, 512], f32r, tag="p1")
                        p_sb = [p_sb0, p_sb1]
                        for j in (0, 1):
                            for ki in (0, 1):
                                kt = g0 + ki
                                m = kt - 4 * qc
                                off = 0 if m < 0 else min(128 * m, 256)
                                nc.tensor.matmul(
                                    s_ps[j][:, ki, ds(off, 512 - off)],
                                    kT[ds(64 * j, 64), ts(kt, 128)],
                                    qT[ds(64 * j, 64), ds(512 * qc + off, 512 - off)],
                                    start=True, stop=True)
                            nc.scalar.activation(
                                p_sb[j][:, :, ds(eoff, 512 - eoff)],
                                s_ps[j][:, :, ds(eoff, 512 - eoff)],
                                Exp, scale=SCALE)
                        for j in (0, 1):
                            for ki in (0, 1):
                                kt = g0 + ki
                                m = kt - 4 * qc
                                if m >= 0:
                                    nc.gpsimd.tensor_mul(
                                        p_sb[j][:, ki, ds(128 * m, 128)],
                                        p_sb[j][:, ki, ds(128 * m, 128)], tri)
                                pvoff = 0 if m < 0 else 128 * m
                                nc.tensor.matmul(
                                    o_ps[j][:, ds(pvoff, 512 - pvoff)],
                                    vaug[:, kt, 2 * p + j, :],
                                    p_sb[j][:, ki, ds(pvoff, 512 - pvoff)],
                                    start=(kt == 0), stop=(kt == nkt - 1))
                    # normalize by softmax denominator (row 64) and accumulate
                    for j in (0, 1):
                        dn = norm.tile([1, 512], f32, tag="dn")
                        nc.vector.tensor_copy(dn, o_ps[j][64:65, :])
                        r = norm.tile([1, 512], f32, tag="r")
                        nc.vector.reciprocal(r, dn)
                        rb = norm.tile([64, 512], f32, tag="rb")
                        nc.gpsimd.partition_broadcast(rb, r)
                        tmp = norm.tile([64, 512], f32, tag="tmp")
                        nc.vector.tensor_mul(tmp, o_ps[j][0:64, :], rb)
                        if p == 0 and j == 0:
                            nc.vector.tensor_copy(yT[:, ts(qc, 512)], tmp)
                        else:
                            nc.vector.tensor_add(yT[:, ts(qc, 512)],
                                                 yT[:, ts(qc, 512)], tmp)

            # ---- Phase 3: ReduceScatter over the core pair, then c_proj
            bounce_in = dram.tile([2, D, T // 2], f32)
            bounce_out = dram.tile([D, T // 2], f32)
            for gg in (0, 1):
                nc.sync.dma_start(bounce_in[gg], yT[:, ds(1024 * gg, 1024)])
            if sim_no_collective:
                nc.sync.dma_start(bounce_out, bounce_in[0])
            else:
                nc.gpsimd.collective_compute(
                    "ReduceScatter", mybir.AluOpType.add,
                    replica_groups=[[0, 1], [2, 3], [4, 5], [6, 7]],
                    ins=[bounce_in.opt()], outs=[bounce_out.opt()])
            ysum = const.tile([D, T // 2], f32r)
            nc.gpsimd.dma_start(ysum, bounce_out)
            cp_cycle = [(ps_x, "px"), (ps_o, "o0"), (ps_o, "o1"), (ps_x, "px")]
            for rt in range(8):
                for nj in (0, 1):
                    pool_, tag_ = cp_cycle[(2 * rt + nj) % 4]
                    cp = pool_.tile([128, 512], f32, tag=tag_)
                    nc.tensor.matmul(cp, ysum[:, ts(rt, 128)],
                                     wp_sb[:, ts(nj, 512)], start=True, stop=True)
                    co = co_pool.tile([128, 512], f32)
                    if nj == 0:
                        nc.vector.tensor_copy(co, cp)
                    else:
                        nc.scalar.copy(co, cp)
                    eng = nc.sync if nj == 0 else nc.scalar
                    eng.dma_start(out_s[ts(rt, 128), ds(512 * nj, 512)], co)

    nc.compile()
    return nc


def _get_nc():
    if "nc" not in _cache:
        _cache["nc"] = _build()
    return _cache["nc"]


def kernel(x, Wq, Wk, Wv, Wp, iter_num=0, trace=False, **_):
    from concourse import bass_utils

    nc = _get_nc()
    x = np.asarray(x, dtype=np.float32)
    Wq = np.asarray(Wq, dtype=np.float32)
    Wk = np.asarray(Wk, dtype=np.float32)
    Wv = np.asarray(Wv, dtype=np.float32)
    Wp = np.asarray(Wp, dtype=np.float32)

    in_maps = []
    for c in range(N_CORES):
        b, g = c // 2, c % 2
        sl = slice(512 * g, 512 * (g + 1))
        in_maps.append({
            "x_s": np.ascontiguousarray(x[b]),
            "wq_s": np.ascontiguousarray(Wq[:, sl]),
            "wk_s": np.ascontiguousarray(Wk[:, sl]),
            "wv_s": np.ascontiguousarray(Wv[:, sl]),
            "wp": np.ascontiguousarray(Wp),
        })
    res = None
    last_err = None
    for _attempt in range(3):
        try:
            res = bass_utils.run_bass_kernel_spmd(nc, in_maps,
                                                  core_ids=list(range(N_CORES)),
                                                  trace=trace)
            break
        except Exception as e:  # transient axon tunnel drops
            last_err = e
    if res is None:
        raise last_err
    out = np.empty((B, T, C), dtype=np.float32)
    for c in range(N_CORES):
        b, g = c // 2, c % 2
        out[b, 1024 * g:1024 * (g + 1), :] = res.results[c]["out_s"]
    if trace:
        return out, res
    return out


# revision 26
# speedup vs baseline: 18888.8672x; 18888.8672x over previous
"""InfiniteHeadAttention Trainium2 kernel (8 NeuronCores).

Reference computation (B=4, T=2048, C=1024, H=16, D=64):
    q,k,v = x@Wq, x@Wk, x@Wv  (per-head split)
    att   = softmax(causal(q k^T / sqrt(D)))
    y     = sum over heads of att@v        # heads SUMMED, not concatenated
    out   = y @ Wp

Sharding: 4-way data-parallel over batch x 2-way over heads.
Core c = 2*b+g handles batch b, heads 8g..8g+7. The per-head y partial sums
are combined with a ReduceScatter over core pairs {2b, 2b+1}; core 2b+g then
applies c_proj to token half g and writes out[b, 1024g:1024(g+1), :].

On-core layout is fully "transposed": projections produce q^T,k^T with the
head dim on partitions, attention computes S^T = K Q^T tiles (128 k-tokens x
512 q-tokens), exp on ScalarE, and P^T V via PSUM accumulation with an extra
ones-column in V producing the softmax denominators for free. Matmuls run in
float32r (1 cycle/row vs 4 for fp32; ~12-13 bit mantissa, plenty for this
problem). Causal masking: lower-triangle k-tiles are skipped outright, the
diagonal tile is masked with a precomputed triangle after exp.
"""

import numpy as np

B, T, C = 4, 2048, 1024
H, D = 16, 64
N_CORES = 8
PAIRS = 4          # head pairs per core (2 heads each)
NCT = C // 128     # c-tiles
NTT = T // 128     # token tiles
NQC = T // 512     # q-chunks
NKT = T // 128     # k-tiles
SCALE = 1.0 / 8.0  # 1/sqrt(D)

_cache = {}


def _build(sim_no_collective=False):
    import concourse.bass as bass
    import concourse.bacc as bacc
    import concourse.tile as tile
    from concourse import mybir
    from concourse.bass import ts, ds
    from concourse.masks import make_identity

    f32, f32r = mybir.dt.float32, mybir.dt.float32r
    Exp = mybir.ActivationFunctionType.Exp

    nc = bacc.Bacc("TRN2", target_bir_lowering=False, debug=False,
                   num_devices=1 if sim_no_collective else N_CORES)

    x_s = nc.dram_tensor("x_s", [T, C], f32r, kind="ExternalInput").ap()
    wq_s = nc.dram_tensor("wq_s", [C, 512], f32r, kind="ExternalInput").ap()
    wk_s = nc.dram_tensor("wk_s", [C, 512], f32r, kind="ExternalInput").ap()
    wv_s = nc.dram_tensor("wv_s", [C, 512], f32r, kind="ExternalInput").ap()
    wp = nc.dram_tensor("wp", [D, C], f32r, kind="ExternalInput").ap()
    out_s = nc.dram_tensor("out_s", [T // 2, C], f32, kind="ExternalOutput").ap()

    with tile.TileContext(nc) as tc:
        with (
            tc.tile_pool(name="const", bufs=1) as const,
            tc.tile_pool(name="xTp", bufs=1) as xTp,
            tc.tile_pool(name="xn", bufs=2) as xn_pool,
            tc.tile_pool(name="wqk", bufs=1) as wqk_pool,
            tc.tile_pool(name="qk", bufs=2) as qk_pool,
            tc.tile_pool(name="pp", bufs=2) as p_pool,
            tc.tile_pool(name="norm", bufs=1) as norm,
            tc.tile_pool(name="co", bufs=2) as co_pool,
            tc.tile_pool(name="dram", bufs=1, space="DRAM") as dram,
            tc.tile_pool(name="ps_s", bufs=1, space="PSUM") as ps_s,
            tc.tile_pool(name="ps_o", bufs=1, space="PSUM") as ps_o,
            tc.tile_pool(name="ps_x", bufs=2, space="PSUM") as ps_x,
        ):
            ident = const.tile([128, 128], f32)
            make_identity(nc, ident)
            identr = const.tile([128, 128], f32r)
            nc.vector.tensor_copy(identr, ident)
            # tri[ik, iq] = 1 if iq >= ik else 0  (valid = k <= q on diagonal tile)
            tri = const.tile([128, 128], f32)
            nc.vector.memset(tri, 1.0)
            nc.gpsimd.affine_select(out=tri, in_=tri,
                                    compare_op=mybir.AluOpType.is_ge,
                                    fill=0.0, base=0, pattern=[[1, 128]],
                                    channel_multiplier=-1)
            ones_c = const.tile([128, 1], f32)
            nc.vector.memset(ones_c, 1.0)

            wp_sb = const.tile([D, C], f32r)
            nc.scalar.dma_start(wp_sb, wp)
            wv_sb = const.tile([128, NCT, 512], f32r)
            nc.scalar.dma_start(wv_sb, wv_s.rearrange("(ct p) d -> p ct d", p=128))

            # v with a ones column appended per head: [tok, kt, head, 65]
            vaug = const.tile([128, NKT, 8, 65], f32r)
            nc.vector.tensor_copy(vaug[:, :, :, 64:65],
                                  ones_c.to_broadcast([128, NKT, 8, 1]))
            yT = const.tile([D, T], f32)

            xT = xTp.tile([128, NCT, T], f32r)

            # ---- Phase 1: transpose x into xT, then v projection per token tile
            for tt in range(NTT):
                xn = xn_pool.tile([128, C], f32r)
                nc.sync.dma_start(xn[:, 0:512], x_s[ts(tt, 128), 0:512])
                nc.scalar.dma_start(xn[:, 512:1024], x_s[ts(tt, 128), 512:1024])
                for half in range(2):
                    tag_ = "s0" if half == 0 else "s1"
                    tp = ps_s.tile([128, 4, 128], f32r, tag=tag_)
                    for ci in range(4):
                        ct = 4 * half + ci
                        nc.tensor.transpose(tp[:, ci, :], xn[:, ts(ct, 128)], identr)
                    if (tt + half) % 2 == 0:
                        nc.scalar.copy(xT[:, ds(4 * half, 4), ts(tt, 128)], tp)
                    else:
                        nc.vector.tensor_copy(xT[:, ds(4 * half, 4), ts(tt, 128)], tp)
                vp = ps_o.tile([128, 512], f32, tag="o0" if tt % 2 == 0 else "o1")
                for ct in range(NCT):
                    nc.tensor.matmul(vp, xT[:, ct, ts(tt, 128)], wv_sb[:, ct, :],
                                     start=(ct == 0), stop=(ct == NCT - 1))
                nc.vector.tensor_copy(vaug[:, tt, :, 0:64],
                                      vp.rearrange("p (h d) -> p h d", h=8))

            # ---- Phase 2: per head-pair projection + attention
            for p in range(PAIRS):
                wq_p = wqk_pool.tile([128, NCT, 128], f32r, tag="wq")
                wk_p = wqk_pool.tile([128, NCT, 128], f32r, tag="wk")
                nc.scalar.dma_start(
                    wq_p, wq_s.rearrange("(ct p) d -> p ct d", p=128)[:, :, ds(128 * p, 128)])
                nc.scalar.dma_start(
                    wk_p, wk_s.rearrange("(ct p) d -> p ct d", p=128)[:, :, ds(128 * p, 128)])
                qT = qk_pool.tile([128, T], f32r, tag="q")
                kT = qk_pool.tile([128, T], f32r, tag="k")
                for tc4 in range(NQC):
                    for w_p, dst in ((wq_p, qT), (wk_p, kT)):
                        pj = ps_x.tile([128, 512], f32, tag="px")
                        for ct in range(NCT):
                            nc.tensor.matmul(pj, w_p[:, ct, :],
                                             xT[:, ct, ts(tc4, 512)],
                                             start=(ct == 0), stop=(ct == NCT - 1))
                        nc.vector.tensor_copy(dst[:, ts(tc4, 512)], pj)

                for qc in range(NQC):
                    o_ps0 = ps_o.tile([65, 512], f32, tag="o0")
                    o_ps1 = ps_o.tile([65, 512], f32, tag="o1")
                    o_ps = [o_ps0, o_ps1]
                    nkt = 4 * qc + 4
                    for g0 in range(0, nkt, 2):
                        m_g0 = g0 - 4 * qc
                        eoff = 256 if m_g0 == 2 else 0
                        s_ps0 = ps_s.tile([128, 2, 512], f32, tag="s0")
                        s_ps1 = ps_s.tile([128, 2, 512], f32, tag="s1")
                        s_ps = [s_ps0, s_ps1]
                        p_sb0 = p_pool.tile([128, 2, 512], f32r, tag="p0")
                        p_sb1 = p_pool.tile([128, 2, 512], f32r, tag="p1")
                        p_sb = [p_sb0, p_sb1]
                        for j in (0, 1):
                            for ki in (0, 1):
                                kt = g0 + ki
                                m = kt - 4 * qc
                                off = 0 if m < 0 else min(128 * m, 256)
                                nc.tensor.matmul(
                                    s_ps[j][:, ki, ds(off, 512 - off)],
                                    kT[ds(64 * j, 64), ts(kt, 128)],
                                    qT[ds(64 * j, 64), ds(512 * qc + off, 512 - off)],
                                    start=True, stop=True)
                            nc.scalar.activation(
                                p_sb[j][:, :, ds(eoff, 512 - eoff)],
                                s_ps[j][:, :, ds(eoff, 512 - eoff)],
                                Exp, scale=SCALE)
                        for j in (0, 1):
                            for ki in (0, 1):
                                kt = g0 + ki
                                m = kt - 4 * qc
                                if m >= 0:
                                    nc.gpsimd.tensor_mul(
                                        p_sb[j][:, ki, ds(128 * m, 128)],
                                        p_sb[j][:, ki, ds(128 * m, 128)], tri)
                                pvoff = 0 if m < 0 else 128 * m
                                nc.tensor.matmul(
                                    o_ps[j][:, ds(pvoff, 512 - pvoff)],
                                    vaug[:, kt, 2 * p + j, :],
                                    p_sb[j][:, ki, ds(pvoff, 512 - pvoff)],
                                    start=(kt == 0), stop=(kt == nkt - 1))
                    # normalize by softmax denominator (row 64) and accumulate
                    for j in (0, 1):
                        dn = norm.tile([1, 512], f32, tag="dn")
                        nc.vector.tensor_copy(dn, o_ps[j][64:65, :])
                        r = norm.tile([1, 512], f32, tag="r")
                        nc.vector.reciprocal(r, dn)
                        rb = norm.tile([64, 512], f32, tag="rb")
                        nc.gpsimd.partition_broadcast(rb, r)
                        tmp = norm.tile([64, 512], f32, tag="tmp")
                        nc.vector.tensor_mul(tmp, o_ps[j][0:64, :], rb)
                        if p == 0 and j == 0:
                            nc.vector.tensor_copy(yT[:, ts(qc, 512)], tmp)
                        else:
                            nc.vector.tensor_add(yT[:, ts(qc, 512)],
                                                 yT[:, ts(qc, 512)], tmp)

            # ---- Phase 3: ReduceScatter over the core pair, then c_proj
            bounce_in_a = dram.tile([2, D, T // 4], f32)
            bounce_in_b = dram.tile([2, D, T // 4], f32)
            bounce_out_a = dram.tile([D, T // 4], f32)
            bounce_out_b = dram.tile([D, T // 4], f32)
            for gg in (0, 1):
                nc.sync.dma_start(bounce_in_a[gg], yT[:, ds(1024 * gg, 512)])
                nc.sync.dma_start(bounce_in_b[gg], yT[:, ds(1024 * gg + 512, 512)])
            rg = [[0, 1], [2, 3], [4, 5], [6, 7]]
            if sim_no_collective:
                nc.sync.dma_start(bounce_out_a, bounce_in_a[0])
                nc.sync.dma_start(bounce_out_b, bounce_in_b[0])
            else:
                nc.gpsimd.collective_compute(
                    "ReduceScatter", mybir.AluOpType.add, replica_groups=rg,
                    ins=[bounce_in_a.opt()], outs=[bounce_out_a.opt()])
                nc.gpsimd.collective_compute(
                    "ReduceScatter", mybir.AluOpType.add, replica_groups=rg,
                    ins=[bounce_in_b.opt()], outs=[bounce_out_b.opt()])
            ysum = const.tile([D, T // 2], f32r)
            nc.gpsimd.dma_start(ysum[:, 0:512], bounce_out_a)
            nc.gpsimd.dma_start(ysum[:, 512:1024], bounce_out_b)
            cp_cycle = [(ps_x, "px"), (ps_o, "o0"), (ps_o, "o1"), (ps_x, "px")]
            for rt in range(8):
                for nj in (0, 1):
                    pool_, tag_ = cp_cycle[(2 * rt + nj) % 4]
                    cp = pool_.tile([128, 512], f32, tag=tag_)
                    nc.tensor.matmul(cp, ysum[:, ts(rt, 128)],
                                     wp_sb[:, ts(nj, 512)], start=True, stop=True)
                    co = co_pool.tile([128, 512], f32)
                    if nj == 0:
                        nc.vector.tensor_copy(co, cp)
                    else:
                        nc.scalar.copy(co, cp)
                    eng = nc.sync if nj == 0 else nc.scalar
                    eng.dma_start(out_s[ts(rt, 128), ds(512 * nj, 512)], co)

    nc.compile()
    return nc


def _get_nc():
    if "nc" not in _cache:
        _cache["nc"] = _build()
    return _cache["nc"]


def kernel(x, Wq, Wk, Wv, Wp, iter_num=0, trace=False, **_):
    from concourse import bass_utils

    nc = _get_nc()
    x = np.asarray(x, dtype=np.float32)
    Wq = np.asarray(Wq, dtype=np.float32)
    Wk = np.asarray(Wk, dtype=np.float32)
    Wv = np.asarray(Wv, dtype=np.float32)
    Wp = np.asarray(Wp, dtype=np.float32)

    in_maps = []
    for c in range(N_CORES):
        b, g = c // 2, c % 2
        sl = slice(512 * g, 512 * (g + 1))
        in_maps.append({
            "x_s": np.ascontiguousarray(x[b]),
            "wq_s": np.ascontiguousarray(Wq[:, sl]),
            "wk_s": np.ascontiguousarray(Wk[:, sl]),
            "wv_s": np.ascontiguousarray(Wv[:, sl]),
            "wp": np.ascontiguousarray(Wp),
        })
    res = None
    last_err = None
    for _attempt in range(3):
        try:
            res = bass_utils.run_bass_kernel_spmd(nc, in_maps,
                                                  core_ids=list(range(N_CORES)),
                                                  trace=trace)
            break
        except Exception as e:  # transient axon tunnel drops
            last_err = e
    if res is None:
        raise last_err
    out = np.empty((B, T, C), dtype=np.float32)
    for c in range(N_CORES):
        b, g = c // 2, c % 2
        out[b, 1024 * g:1024 * (g + 1), :] = res.results[c]["out_s"]
    if trace:
        return out, res
    return out


# revision 30
# speedup vs baseline: 19067.1953x; 1.0094x over previous
"""InfiniteHeadAttention Trainium2 kernel (8 NeuronCores).

Reference computation (B=4, T=2048, C=1024, H=16, D=64):
    q,k,v = x@Wq, x@Wk, x@Wv  (per-head split)
    att   = softmax(causal(q k^T / sqrt(D)))
    y     = sum over heads of att@v        # heads SUMMED, not concatenated
    out   = y @ Wp

Sharding: 4-way data-parallel over batch x 2-way over heads.
Core c = 2*b+g handles batch b, heads 8g..8g+7. The per-head y partial sums
are combined with a ReduceScatter over core pairs {2b, 2b+1}; core 2b+g then
applies c_proj to token half g and writes out[b, 1024g:1024(g+1), :].

On-core layout is fully "transposed": projections produce q^T,k^T with the
head dim on partitions, attention computes S^T = K Q^T tiles (128 k-tokens x
512 q-tokens), exp on ScalarE, and P^T V via PSUM accumulation with an extra
ones-column in V producing the softmax denominators for free. Matmuls run in
float32r (1 cycle/row vs 4 for fp32; ~12-13 bit mantissa, plenty for this
problem). Causal masking: lower-triangle k-tiles are skipped outright, the
diagonal tile is masked with a precomputed triangle after exp.
"""

import numpy as np

B, T, C = 4, 2048, 1024
H, D = 16, 64
N_CORES = 8
PAIRS = 4          # head pairs per core (2 heads each)
NCT = C // 128     # c-tiles
NTT = T // 128     # token tiles
NQC = T // 512     # q-chunks
NKT = T // 128     # k-tiles
SCALE = 1.0 / 8.0  # 1/sqrt(D)

_cache = {}


def _build(sim_no_collective=False):
    import concourse.bass as bass
    import concourse.bacc as bacc
    import concourse.tile as tile
    from concourse import mybir
    from concourse.bass import ts, ds
    from concourse.masks import make_identity

    f32, f32r = mybir.dt.float32, mybir.dt.float32r
    Exp = mybir.ActivationFunctionType.Exp

    nc = bacc.Bacc("TRN2", target_bir_lowering=False, debug=False,
                   num_devices=1 if sim_no_collective else N_CORES)

    x_s = nc.dram_tensor("x_s", [T, C], f32r, kind="ExternalInput").ap()
    wq_s = nc.dram_tensor("wq_s", [C, 512], f32r, kind="ExternalInput").ap()
    wk_s = nc.dram_tensor("wk_s", [C, 512], f32r, kind="ExternalInput").ap()
    wv_s = nc.dram_tensor("wv_s", [C, 512], f32r, kind="ExternalInput").ap()
    wp = nc.dram_tensor("wp", [D, C], f32r, kind="ExternalInput").ap()
    out_s = nc.dram_tensor("out_s", [T // 2, C], f32, kind="ExternalOutput").ap()

    with tile.TileContext(nc) as tc:
        with (
            tc.tile_pool(name="const", bufs=1) as const,
            tc.tile_pool(name="xTp", bufs=1) as xTp,
            tc.tile_pool(name="xn", bufs=2) as xn_pool,
            tc.tile_pool(name="wqk", bufs=1) as wqk_pool,
            tc.tile_pool(name="qk", bufs=2) as qk_pool,
            tc.tile_pool(name="pp", bufs=2) as p_pool,
            tc.tile_pool(name="norm", bufs=1) as norm,
            tc.tile_pool(name="co", bufs=2) as co_pool,
            tc.tile_pool(name="dram", bufs=1, space="DRAM") as dram,
            tc.tile_pool(name="ps_s", bufs=1, space="PSUM") as ps_s,
            tc.tile_pool(name="ps_o", bufs=1, space="PSUM") as ps_o,
            tc.tile_pool(name="ps_x", bufs=2, space="PSUM") as ps_x,
        ):
            ident = const.tile([128, 128], f32)
            make_identity(nc, ident)
            identr = const.tile([128, 128], f32r)
            nc.vector.tensor_copy(identr, ident)
            # tri[ik, iq] = 1 if iq >= ik else 0  (valid = k <= q on diagonal tile)
            tri = const.tile([128, 128], f32)
            nc.vector.memset(tri, 1.0)
            nc.gpsimd.affine_select(out=tri, in_=tri,
                                    compare_op=mybir.AluOpType.is_ge,
                                    fill=0.0, base=0, pattern=[[1, 128]],
                                    channel_multiplier=-1)
            ones_c = const.tile([128, 1], f32)
            nc.vector.memset(ones_c, 1.0)

            wp_sb = const.tile([D, C], f32r)
            nc.scalar.dma_start(wp_sb, wp)
            wv_sb = const.tile([128, NCT, 512], f32r)
            nc.scalar.dma_start(wv_sb, wv_s.rearrange("(ct p) d -> p ct d", p=128))

            # v with a ones column appended per head: [tok, kt, head, 65]
            vaug = const.tile([128, NKT, 8, 65], f32r)
            nc.vector.tensor_copy(vaug[:, :, :, 64:65],
                                  ones_c.to_broadcast([128, NKT, 8, 1]))
            yT = const.tile([D, T], f32)

            xT = xTp.tile([128, NCT, T], f32r)

            # ---- Phase 1a: transpose x into xT (tight PE/copy pipeline)
            for tt in range(NTT):
                xn = xn_pool.tile([128, C], f32r)
                nc.sync.dma_start(xn[:, 0:512], x_s[ts(tt, 128), 0:512])
                nc.scalar.dma_start(xn[:, 512:1024], x_s[ts(tt, 128), 512:1024])
                for half in range(2):
                    tag_ = "s0" if half == 0 else "s1"
                    tp = ps_s.tile([128, 4, 128], f32r, tag=tag_)
                    for ci in range(4):
                        ct = 4 * half + ci
                        nc.tensor.transpose(tp[:, ci, :], xn[:, ts(ct, 128)], identr)
                    if (tt + half) % 2 == 0:
                        nc.scalar.copy(xT[:, ds(4 * half, 4), ts(tt, 128)], tp)
                    else:
                        nc.vector.tensor_copy(xT[:, ds(4 * half, 4), ts(tt, 128)], tp)
            # ---- Phase 1b: v projection (PE-bound, 4-deep PSUM rotation)
            for tt in range(NTT):
                vtag = ["o0", "o1", "px", "px"][tt % 4]
                vpool = {"o0": ps_o, "o1": ps_o, "px": ps_x}[vtag]
                vp = vpool.tile([128, 512], f32, tag=vtag)
                for ct in range(NCT):
                    nc.tensor.matmul(vp, xT[:, ct, ts(tt, 128)], wv_sb[:, ct, :],
                                     start=(ct == 0), stop=(ct == NCT - 1))
                if tt % 2 == 0:
                    nc.vector.tensor_copy(vaug[:, tt, :, 0:64],
                                          vp.rearrange("p (h d) -> p h d", h=8))
                else:
                    nc.scalar.copy(vaug[:, tt, :, 0:64],
                                   vp.rearrange("p (h d) -> p h d", h=8))

            # ---- Phase 2: per head-pair projection + attention
            for p in range(PAIRS):
                wq_p = wqk_pool.tile([128, NCT, 128], f32r, tag="wq")
                wk_p = wqk_pool.tile([128, NCT, 128], f32r, tag="wk")
                nc.scalar.dma_start(
                    wq_p, wq_s.rearrange("(ct p) d -> p ct d", p=128)[:, :, ds(128 * p, 128)])
                nc.scalar.dma_start(
                    wk_p, wk_s.rearrange("(ct p) d -> p ct d", p=128)[:, :, ds(128 * p, 128)])
                qT = qk_pool.tile([128, T], f32r, tag="q")
                kT = qk_pool.tile([128, T], f32r, tag="k")
                for tc4 in range(NQC):
                    for w_p, dst in ((wq_p, qT), (wk_p, kT)):
                        pj = ps_x.tile([128, 512], f32, tag="px")
                        for ct in range(NCT):
                            nc.tensor.matmul(pj, w_p[:, ct, :],
                                             xT[:, ct, ts(tc4, 512)],
                                             start=(ct == 0), stop=(ct == NCT - 1))
                        nc.vector.tensor_copy(dst[:, ts(tc4, 512)], pj)

                for qc in range(NQC):
                    o_ps0 = ps_o.tile([65, 512], f32, tag="o0")
                    o_ps1 = ps_o.tile([65, 512], f32, tag="o1")
                    o_ps = [o_ps0, o_ps1]
                    nkt = 4 * qc + 4
                    for g0 in range(0, nkt, 2):
                        m_g0 = g0 - 4 * qc
                        eoff = 256 if m_g0 == 2 else 0
                        s_ps0 = ps_s.tile([128, 2, 512], f32, tag="s0")
                        s_ps1 = ps_s.tile([128, 2, 512], f32, tag="s1")
                        s_ps = [s_ps0, s_ps1]
                        p_sb0 = p_pool.tile([128, 2, 512], f32r, tag="p0")
                        p_sb1 = p_pool.tile([128, 2, 512], f32r, tag="p1")
                        p_sb = [p_sb0, p_sb1]
                        for j in (0, 1):
                            for ki in (0, 1):
                                kt = g0 + ki
                                m = kt - 4 * qc
                                off = 0 if m < 0 else min(128 * m, 256)
                                nc.tensor.matmul(
                                    s_ps[j][:, ki, ds(off, 512 - off)],
                                    kT[ds(64 * j, 64), ts(kt, 128)],
                                    qT[ds(64 * j, 64), ds(512 * qc + off, 512 - off)],
                                    start=True, stop=True)
                            nc.scalar.activation(
                                p_sb[j][:, :, ds(eoff, 512 - eoff)],
                                s_ps[j][:, :, ds(eoff, 512 - eoff)],
                                Exp, scale=SCALE)
                        for j in (0, 1):
                            for ki in (0, 1):
                                kt = g0 + ki
                                m = kt - 4 * qc
                                if m >= 0:
                                    meng = nc.gpsimd if (m + j) % 2 == 0 else nc.vector
                                    meng.tensor_mul(
                                        p_sb[j][:, ki, ds(128 * m, 128)],
                                        p_sb[j][:, ki, ds(128 * m, 128)], tri)
                                pvoff = 0 if m < 0 else 128 * m
                                nc.tensor.matmul(
                                    o_ps[j][:, ds(pvoff, 512 - pvoff)],
                                    vaug[:, kt, 2 * p + j, :],
                                    p_sb[j][:, ki, ds(pvoff, 512 - pvoff)],
                                    start=(kt == 0), stop=(kt == nkt - 1))
                    # normalize by softmax denominator (row 64) and accumulate
                    for j in (0, 1):
                        dn = norm.tile([1, 512], f32, tag="dn")
                        nc.vector.tensor_copy(dn, o_ps[j][64:65, :])
                        r = norm.tile([1, 512], f32, tag="r")
                        nc.vector.reciprocal(r, dn)
                        rb = norm.tile([64, 512], f32, tag="rb")
                        nc.gpsimd.partition_broadcast(rb, r)
                        tmp = norm.tile([64, 512], f32, tag="tmp")
                        nc.vector.tensor_mul(tmp, o_ps[j][0:64, :], rb)
                        if p == 0 and j == 0:
                            nc.vector.tensor_copy(yT[:, ts(qc, 512)], tmp)
                        else:
                            nc.vector.tensor_add(yT[:, ts(qc, 512)],
                                                 yT[:, ts(qc, 512)], tmp)

            # ---- Phase 3: ReduceScatter over the core pair, then c_proj
            bounce_in_a = dram.tile([2, D, T // 4], f32)
            bounce_in_b = dram.tile([2, D, T // 4], f32)
            bounce_out_a = dram.tile([D, T // 4], f32)
            bounce_out_b = dram.tile([D, T // 4], f32)
            for gg in (0, 1):
                nc.sync.dma_start(bounce_in_a[gg], yT[:, ds(1024 * gg, 512)])
                nc.sync.dma_start(bounce_in_b[gg], yT[:, ds(1024 * gg + 512, 512)])
            rg = [[0, 1], [2, 3], [4, 5], [6, 7]]
            if sim_no_collective:
                nc.sync.dma_start(bounce_out_a, bounce_in_a[0])
                nc.sync.dma_start(bounce_out_b, bounce_in_b[0])
            else:
                nc.gpsimd.collective_compute(
                    "ReduceScatter", mybir.AluOpType.add, replica_groups=rg,
                    ins=[bounce_in_a.opt()], outs=[bounce_out_a.opt()])
                nc.gpsimd.collective_compute(
                    "ReduceScatter", mybir.AluOpType.add, replica_groups=rg,
                    ins=[bounce_in_b.opt()], outs=[bounce_out_b.opt()])
            ysum = const.tile([D, T // 2], f32r)
            nc.gpsimd.dma_start(ysum[:, 0:512], bounce_out_a)
            nc.gpsimd.dma_start(ysum[:, 512:1024], bounce_out_b)
            cp_cycle = [(ps_x, "px"), (ps_o, "o0"), (ps_o, "o1"), (ps_x, "px")]
            for rt in range(8):
                for nj in (0, 1):
                    pool_, tag_ = cp_cycle[(2 * rt + nj) % 4]
                    cp = pool_.tile([128, 512], f32, tag=tag_)
                    nc.tensor.matmul(cp, ysum[:, ts(rt, 128)],
                                     wp_sb[:, ts(nj, 512)], start=True, stop=True)
                    co = co_pool.tile([128, 512], f32)
                    if nj == 0:
                        nc.vector.tensor_copy(co, cp)
                    else:
                        nc.scalar.copy(co, cp)
                    eng = nc.sync if nj == 0 else nc.scalar
                    eng.dma_start(out_s[ts(rt, 128), ds(512 * nj, 512)], co)

    nc.compile()
    return nc


def _get_nc():
    if "nc" not in _cache:
        _cache["nc"] = _build()
    return _cache["nc"]


def kernel(x, Wq, Wk, Wv, Wp, iter_num=0, trace=False, **_):
    from concourse import bass_utils

    nc = _get_nc()
    x = np.asarray(x, dtype=np.float32)
    Wq = np.asarray(Wq, dtype=np.float32)
    Wk = np.asarray(Wk, dtype=np.float32)
    Wv = np.asarray(Wv, dtype=np.float32)
    Wp = np.asarray(Wp, dtype=np.float32)

    in_maps = []
    for c in range(N_CORES):
        b, g = c // 2, c % 2
        sl = slice(512 * g, 512 * (g + 1))
        in_maps.append({
            "x_s": np.ascontiguousarray(x[b]),
            "wq_s": np.ascontiguousarray(Wq[:, sl]),
            "wk_s": np.ascontiguousarray(Wk[:, sl]),
            "wv_s": np.ascontiguousarray(Wv[:, sl]),
            "wp": np.ascontiguousarray(Wp),
        })
    res = None
    last_err = None
    for _attempt in range(3):
        try:
            res = bass_utils.run_bass_kernel_spmd(nc, in_maps,
                                                  core_ids=list(range(N_CORES)),
                                                  trace=trace)
            break
        except Exception as e:  # transient axon tunnel drops
            last_err = e
    if res is None:
        raise last_err
    out = np.empty((B, T, C), dtype=np.float32)
    for c in range(N_CORES):
        b, g = c // 2, c % 2
        out[b, 1024 * g:1024 * (g + 1), :] = res.results[c]["out_s"]
    if trace:
        return out, res
    return out


# revision 35
# speedup vs baseline: 19310.0554x; 1.0127x over previous
"""InfiniteHeadAttention Trainium2 kernel (8 NeuronCores).

Reference computation (B=4, T=2048, C=1024, H=16, D=64):
    q,k,v = x@Wq, x@Wk, x@Wv  (per-head split)
    att   = softmax(causal(q k^T / sqrt(D)))
    y     = sum over heads of att@v        # heads SUMMED, not concatenated
    out   = y @ Wp

Sharding: 4-way data-parallel over batch x 2-way over heads.
Core c = 2*b+g handles batch b, heads 8g..8g+7. The per-head y partial sums
are combined with a ReduceScatter over core pairs {2b, 2b+1}; core 2b+g then
applies c_proj to token half g and writes out[b, 1024g:1024(g+1), :].

On-core layout is fully "transposed": projections produce q^T,k^T with the
head dim on partitions, attention computes S^T = K Q^T tiles (128 k-tokens x
512 q-tokens), exp on ScalarE, and P^T V via PSUM accumulation with an extra
ones-column in V producing the softmax denominators for free. Matmuls run in
float32r (1 cycle/row vs 4 for fp32; ~12-13 bit mantissa, plenty for this
problem). Causal masking: lower-triangle k-tiles are skipped outright, the
diagonal tile is masked with a precomputed triangle after exp.
"""

import numpy as np

B, T, C = 4, 2048, 1024
H, D = 16, 64
N_CORES = 8
PAIRS = 4          # head pairs per core (2 heads each)
NCT = C // 128     # c-tiles
NTT = T // 128     # token tiles
NQC = T // 512     # q-chunks
NKT = T // 128     # k-tiles
SCALE = 1.0 / 8.0  # 1/sqrt(D)

_cache = {}


def _build(sim_no_collective=False):
    import concourse.bass as bass
    import concourse.bacc as bacc
    import concourse.tile as tile
    from concourse import mybir
    from concourse.bass import ts, ds
    from concourse.masks import make_identity

    f32, f32r = mybir.dt.float32, mybir.dt.float32r
    Exp = mybir.ActivationFunctionType.Exp

    nc = bacc.Bacc("TRN2", target_bir_lowering=False, debug=False,
                   num_devices=1 if sim_no_collective else N_CORES)

    x_s = nc.dram_tensor("x_s", [T, C], f32r, kind="ExternalInput").ap()
    wq_s = nc.dram_tensor("wq_s", [C, 512], f32r, kind="ExternalInput").ap()
    wk_s = nc.dram_tensor("wk_s", [C, 512], f32r, kind="ExternalInput").ap()
    wv_s = nc.dram_tensor("wv_s", [C, 512], f32r, kind="ExternalInput").ap()
    wp = nc.dram_tensor("wp", [D, C], f32r, kind="ExternalInput").ap()
    out_s = nc.dram_tensor("out_s", [T // 2, C], f32, kind="ExternalOutput").ap()

    with tile.TileContext(nc) as tc:
        with (
            tc.tile_pool(name="const", bufs=1) as const,
            tc.tile_pool(name="xTp", bufs=1) as xTp,
            tc.tile_pool(name="xn", bufs=2) as xn_pool,
            tc.tile_pool(name="wqk", bufs=1) as wqk_pool,
            tc.tile_pool(name="qk", bufs=2) as qk_pool,
            tc.tile_pool(name="pp", bufs=2) as p_pool,
            tc.tile_pool(name="norm", bufs=1) as norm,
            tc.tile_pool(name="co", bufs=2) as co_pool,
            tc.tile_pool(name="dram", bufs=1, space="DRAM") as dram,
            tc.tile_pool(name="ps_s", bufs=1, space="PSUM") as ps_s,
            tc.tile_pool(name="ps_o", bufs=1, space="PSUM") as ps_o,
            tc.tile_pool(name="ps_x", bufs=2, space="PSUM") as ps_x,
        ):
            ident = const.tile([128, 128], f32)
            make_identity(nc, ident)
            identr = const.tile([128, 128], f32r)
            nc.vector.tensor_copy(identr, ident)
            # tri[ik, iq] = 1 if iq >= ik else 0  (valid = k <= q on diagonal tile)
            tri = const.tile([128, 128], f32)
            nc.vector.memset(tri, 1.0)
            nc.gpsimd.affine_select(out=tri, in_=tri,
                                    compare_op=mybir.AluOpType.is_ge,
                                    fill=0.0, base=0, pattern=[[1, 128]],
                                    channel_multiplier=-1)
            ones_c = const.tile([128, 1], f32)
            nc.vector.memset(ones_c, 1.0)

            wp_sb = const.tile([D, C], f32r)
            wv_sb = const.tile([128, NCT, 512], f32r)

            # v with a ones column appended per head: [tok, kt, head, 65]
            vaug = const.tile([128, NKT, 8, 65], f32r)
            nc.vector.tensor_copy(vaug[:, :, :, 64:65],
                                  ones_c.to_broadcast([128, NKT, 8, 1]))
            yT = const.tile([D, T], f32)

            xT = xTp.tile([128, NCT, T], f32r)

            # ---- Phase 1a: transpose x into xT (tight PE/copy pipeline)
            for tt in range(NTT):
                xn = xn_pool.tile([128, C], f32r)
                nc.sync.dma_start(xn[:, 0:512], x_s[ts(tt, 128), 0:512])
                nc.scalar.dma_start(xn[:, 512:1024], x_s[ts(tt, 128), 512:1024])
                for half in range(2):
                    tag_ = "s0" if half == 0 else "s1"
                    tp = ps_s.tile([128, 4, 128], f32r, tag=tag_)
                    for ci in range(4):
                        ct = 4 * half + ci
                        nc.tensor.transpose(tp[:, ci, :], xn[:, ts(ct, 128)], identr)
                    if (tt + half) % 2 == 0:
                        nc.scalar.copy(xT[:, ds(4 * half, 4), ts(tt, 128)], tp)
                    else:
                        nc.vector.tensor_copy(xT[:, ds(4 * half, 4), ts(tt, 128)], tp)
            nc.scalar.dma_start(wv_sb, wv_s.rearrange("(ct p) d -> p ct d", p=128))
            nc.scalar.dma_start(wp_sb, wp)
            # ---- Phase 1b: v projection (PE-bound, 4-deep PSUM rotation)
            for tt in range(NTT):
                vtag = ["o0", "o1", "px", "px"][tt % 4]
                vpool = {"o0": ps_o, "o1": ps_o, "px": ps_x}[vtag]
                vp = vpool.tile([128, 512], f32, tag=vtag)
                for ct in range(NCT):
                    nc.tensor.matmul(vp, xT[:, ct, ts(tt, 128)], wv_sb[:, ct, :],
                                     start=(ct == 0), stop=(ct == NCT - 1))
                if tt % 2 == 0:
                    nc.vector.tensor_copy(vaug[:, tt, :, 0:64],
                                          vp.rearrange("p (h d) -> p h d", h=8))
                else:
                    nc.scalar.copy(vaug[:, tt, :, 0:64],
                                   vp.rearrange("p (h d) -> p h d", h=8))

            # ---- Phase 2: per head-pair projection + attention
            for p in range(PAIRS):
                wq_p = wqk_pool.tile([128, NCT, 128], f32r, tag="wq")
                wk_p = wqk_pool.tile([128, NCT, 128], f32r, tag="wk")
                nc.scalar.dma_start(
                    wq_p, wq_s.rearrange("(ct p) d -> p ct d", p=128)[:, :, ds(128 * p, 128)])
                nc.scalar.dma_start(
                    wk_p, wk_s.rearrange("(ct p) d -> p ct d", p=128)[:, :, ds(128 * p, 128)])
                qT = qk_pool.tile([128, T], f32r, tag="q")
                kT = qk_pool.tile([128, T], f32r, tag="k")
                for tc4 in range(NQC):
                    for w_p, dst in ((wq_p, qT), (wk_p, kT)):
                        pj = ps_x.tile([128, 512], f32, tag="px")
                        for ct in range(NCT):
                            nc.tensor.matmul(pj, w_p[:, ct, :],
                                             xT[:, ct, ts(tc4, 512)],
                                             start=(ct == 0), stop=(ct == NCT - 1))
                        nc.vector.tensor_copy(dst[:, ts(tc4, 512)], pj)

                for qc in range(NQC):
                    o_ps0 = ps_o.tile([65, 512], f32, tag="o0")
                    o_ps1 = ps_o.tile([65, 512], f32, tag="o1")
                    o_ps = [o_ps0, o_ps1]
                    nkt = 4 * qc + 4
                    for g0 in range(0, nkt, 2):
                        m_g0 = g0 - 4 * qc
                        eoff = 256 if m_g0 == 2 else 0
                        s_ps0 = ps_s.tile([128, 2, 512], f32, tag="s0")
                        s_ps1 = ps_s.tile([128, 2, 512], f32, tag="s1")
                        s_ps = [s_ps0, s_ps1]
                        p_sb0 = p_pool.tile([128, 2, 512], f32r, tag="p0")
                        p_sb1 = p_pool.tile([128, 2, 512], f32r, tag="p1")
                        p_sb = [p_sb0, p_sb1]
                        for j in (0, 1):
                            for ki in (0, 1):
                                kt = g0 + ki
                                m = kt - 4 * qc
                                off = 0 if m < 0 else min(128 * m, 256)
                                nc.tensor.matmul(
                                    s_ps[j][:, ki, ds(off, 512 - off)],
                                    kT[ds(64 * j, 64), ts(kt, 128)],
                                    qT[ds(64 * j, 64), ds(512 * qc + off, 512 - off)],
                                    start=True, stop=True)
                            nc.scalar.activation(
                                p_sb[j][:, :, ds(eoff, 512 - eoff)],
                                s_ps[j][:, :, ds(eoff, 512 - eoff)],
                                Exp, scale=SCALE)
                        for j in (0, 1):
                            for ki in (0, 1):
                                kt = g0 + ki
                                m = kt - 4 * qc
                                if m >= 0:
                                    meng = nc.gpsimd if (m + j) % 2 == 0 else nc.vector
                                    meng.tensor_mul(
                                        p_sb[j][:, ki, ds(128 * m, 128)],
                                        p_sb[j][:, ki, ds(128 * m, 128)], tri)
                                pvoff = 0 if m < 0 else 128 * m
                                nc.tensor.matmul(
                                    o_ps[j][:, ds(pvoff, 512 - pvoff)],
                                    vaug[:, kt, 2 * p + j, :],
                                    p_sb[j][:, ki, ds(pvoff, 512 - pvoff)],
                                    start=(kt == 0), stop=(kt == nkt - 1))
                    # normalize by softmax denominator (row 64) and accumulate
                    for j in (0, 1):
                        dn = norm.tile([1, 512], f32, tag="dn")
                        nc.vector.tensor_copy(dn, o_ps[j][64:65, :])
                        r = norm.tile([1, 512], f32, tag="r")
                        nc.vector.reciprocal(r, dn)
                        rb = norm.tile([64, 512], f32, tag="rb")
                        nc.gpsimd.partition_broadcast(rb, r)
                        tmp = norm.tile([64, 512], f32, tag="tmp")
                        nc.vector.tensor_mul(tmp, o_ps[j][0:64, :], rb)
                        if p == 0 and j == 0:
                            nc.vector.tensor_copy(yT[:, ts(qc, 512)], tmp)
                        else:
                            nc.vector.tensor_add(yT[:, ts(qc, 512)],
                                                 yT[:, ts(qc, 512)], tmp)

            # ---- Phase 3: ReduceScatter over the core pair, then c_proj
            bounce_in_a = dram.tile([2, D, T // 4], f32)
            bounce_in_b = dram.tile([2, D, T // 4], f32)
            bounce_out_a = dram.tile([D, T // 4], f32)
            bounce_out_b = dram.tile([D, T // 4], f32)
            for gg in (0, 1):
                nc.sync.dma_start(bounce_in_a[gg], yT[:, ds(1024 * gg, 512)])
                nc.sync.dma_start(bounce_in_b[gg], yT[:, ds(1024 * gg + 512, 512)])
            rg = [[0, 1], [2, 3], [4, 5], [6, 7]]
            if sim_no_collective:
                nc.sync.dma_start(bounce_out_a, bounce_in_a[0])
                nc.sync.dma_start(bounce_out_b, bounce_in_b[0])
            else:
                nc.gpsimd.collective_compute(
                    "ReduceScatter", mybir.AluOpType.add, replica_groups=rg,
                    ins=[bounce_in_a.opt()], outs=[bounce_out_a.opt()])
                nc.gpsimd.collective_compute(
                    "ReduceScatter", mybir.AluOpType.add, replica_groups=rg,
                    ins=[bounce_in_b.opt()], outs=[bounce_out_b.opt()])
            ysum = const.tile([D, T // 2], f32r)
            nc.gpsimd.dma_start(ysum[:, 0:512], bounce_out_a)
            nc.gpsimd.dma_start(ysum[:, 512:1024], bounce_out_b)
            cp_cycle = [(ps_x, "px"), (ps_o, "o0"), (ps_o, "o1"), (ps_x, "px")]
            for rt in range(8):
                for nj in (0, 1):
                    pool_, tag_ = cp_cycle[(2 * rt + nj) % 4]
                    cp = pool_.tile([128, 512], f32, tag=tag_)
                    nc.tensor.matmul(cp, ysum[:, ts(rt, 128)],
                                     wp_sb[:, ts(nj, 512)], start=True, stop=True)
                    co = co_pool.tile([128, 512], f32)
                    if nj == 0:
                        nc.vector.tensor_copy(co, cp)
                    else:
                        nc.scalar.copy(co, cp)
                    eng = nc.sync if nj == 0 else nc.scalar
                    eng.dma_start(out_s[ts(rt, 128), ds(512 * nj, 512)], co)

    nc.compile()
    return nc


def _get_nc():
    if "nc" not in _cache:
        _cache["nc"] = _build()
    return _cache["nc"]


def kernel(x, Wq, Wk, Wv, Wp, iter_num=0, trace=False, **_):
    from concourse import bass_utils

    nc = _get_nc()
    x = np.asarray(x, dtype=np.float32)
    Wq = np.asarray(Wq, dtype=np.float32)
    Wk = np.asarray(Wk, dtype=np.float32)
    Wv = np.asarray(Wv, dtype=np.float32)
    Wp = np.asarray(Wp, dtype=np.float32)

    in_maps = []
    for c in range(N_CORES):
        b, g = c // 2, c % 2
        sl = slice(512 * g, 512 * (g + 1))
        in_maps.append({
            "x_s": np.ascontiguousarray(x[b]),
            "wq_s": np.ascontiguousarray(Wq[:, sl]),
            "wk_s": np.ascontiguousarray(Wk[:, sl]),
            "wv_s": np.ascontiguousarray(Wv[:, sl]),
            "wp": np.ascontiguousarray(Wp),
        })
    res = None
    last_err = None
    for _attempt in range(3):
        try:
            res = bass_utils.run_bass_kernel_spmd(nc, in_maps,
                                                  core_ids=list(range(N_CORES)),
                                                  trace=trace)
            break
        except Exception as e:  # transient axon tunnel drops
            last_err = e
    if res is None:
        raise last_err
    out = np.empty((B, T, C), dtype=np.float32)
    for c in range(N_CORES):
        b, g = c // 2, c % 2
        out[b, 1024 * g:1024 * (g + 1), :] = res.results[c]["out_s"]
    if trace:
        return out, res
    return out
